# revision 24
# baseline (speedup 1.0000x reference)
"""De-stationary causal attention (B=2, L=S=2048, H=8, E=64) on 8 TRN2 cores.

Sharding: the 16 (batch, head) pairs are distributed 2-per-core (cores 0-3
get batch 0, heads 0..7; cores 4-7 get batch 1). Each core runs the same
Bass program on its two pairs.

Math: logits = (Q K^T) * (tau/sqrt(E)) + delta/sqrt(E), causal softmax, A V.
Host-side folds: Q is pre-scaled by tau/sqrt(E); exp(delta/sqrt(E)) is folded
into V (and into the appended denominator column), because
softmax(x + d)_s = exp(x_s) e^{d_s} / sum_j exp(x_j) e^{d_j}.

Device structure per (b,h) pair, scores kept TRANSPOSED (s on partitions):
the causal lower triangle is tiled into 128(s)x{128..512}(l) chunks packed
into 512-col PSUM "slots" (1 bank each).  Slots are grouped 3 per ST buffer
([128,1536] = 3 banks, double buffered) and each buffer gets ONE big exp
on the scalar engine -- the ACT engine is the roofline for this kernel, so
activation count/width is what everything else is shaped around.  The
causal triangle inside diagonal 128-blocks is handled by ACCUMULATING a
-1e4 triangle into PSUM right after the score matmul (two concurrent k=64
identity matmuls on partition halves, bf16), so exp() underflows to exactly
0 and no element-wise masking is needed anywhere.  AV matmuls then stream
the packed A segments into a per-bank [65,512] PSUM accumulator (col 64 is
the softmax denominator, via a ones-ish column folded into V).  Each bank
is copied to SBUF and DMA'd out UN-normalized and transposed; the host does
out[l,e] = ot[e,l] / ot[64,l] (O(L*E) work, off the device critical path).

A dense burst of back-to-back matmuls at the start (alternating PE row
halves so LDWEIGHTS pipelines) flips the PE HAM clock gate to 2.4 GHz
before real work begins.
"""

import copy
import sys

import numpy as np

try:
    import concourse.bass as bass
except ImportError:  # pragma: no cover
    sys.path.insert(0, "/opt/trn_rl_repo")
    import concourse.bass as bass

import concourse.mybir as mybir
import concourse.tile as tile
from concourse.bass_utils import run_bass_kernel_spmd
from concourse.vector_clock import ScopedClock

B, L, H, E = 2, 2048, 8, 64
N_CORES = 8
PAIRS_PER_CORE = 2
SCALE = 1.0 / np.sqrt(np.float32(E))  # 0.125
NEG = -1.0e4  # causal-mask addend; exp(NEG + score) == 0.0 in fp32

f32 = mybir.dt.float32
f32r = mybir.dt.float32r
bf16 = mybir.dt.bfloat16

NT = L // 128  # 16 s-tiles / l-tiles
NB = L // 512  # 4 output banks of 512 l-values
VW = E + 2  # v row: 64 values + denominator col + pad
SLOTS_PER_BUF = 3  # 3 psum banks per ST buffer
N_WARM = 20  # HAM warm-up matmuls

# ---------------------------------------------------------------------------
# Walrus in this toolchain rejects >1 sync-wait per instruction. Split extra
# waits onto NoOps committed just before the instruction on the same engine.
# ---------------------------------------------------------------------------
_NOP_TEMPLATE = {}


def _make_nop(engine, name):
    if engine not in _NOP_TEMPLATE:
        tmp = bass.Bass()
        _NOP_TEMPLATE[engine] = tmp.engines[engine].nop(nofuse=True).ins
    nop = copy.copy(_NOP_TEMPLATE[engine])
    nop.name = name
    nop.engine = engine
    nop.sync_info = None
    return nop


class SplitWaitTileContext(tile.TileContext):
    _ws_counter = 0

    def _split_waits(self, inst):
        si = inst.sync_info
        if si is None or not si.on_wait or len(si.on_wait) <= 1:
            return []
        if inst.engine == mybir.EngineType.Unassigned:
            return []
        waits = list(si.on_wait)
        inst.sync_info = mybir.SyncInfo(
            on_wait=[waits[0]], on_update=list(si.on_update or [])
        )
        nops = []
        for w in waits[1:]:
            SplitWaitTileContext._ws_counter += 1
            nop = _make_nop(inst.engine, f"I-ws{SplitWaitTileContext._ws_counter}")
            nop.sync_info = mybir.SyncInfo(on_wait=[w], on_update=[])
            nops.append(nop)
        return nops

    def _commit_instruction(self, inst, lazy_reg_writes=True):
        for nop in self._split_waits(inst):
            self._add_instruction(nop)
        super()._commit_instruction(inst, lazy_reg_writes)

    def _drain_and_barrier(self, tick_clock, wait_clock):
        nc = self.nc
        probe = nc.sync.nop(nofuse=True)
        wait_clock.add_sem_waits(
            probe.ins, ScopedClock({None: tick_clock.global_clock})
        )
        waits = list(probe.ins.sync_info.on_wait or []) if probe.ins.sync_info else []
        if len(waits) > 1:
            probe.ins.sync_info.on_wait = [waits[0]]
            handles = {h.num: h for h in self.sems.allocated().values()}
            for w in waits[1:]:
                nop = nc.sync.nop(nofuse=True)
                nop.wait_op(handles[w.id], w.wait_value, "sem-ge")
        nc.sync.drain()

        nc.all_engine_barrier()
        assert self.sems is not None
        popped = nc._tile_sem_poison_stack.pop()
        assert popped is self._sem_poison
        nc.clear_and_free_semaphores(list(self.sems.allocated().values()))
        nc.all_engine_barrier()


# ---------------------------------------------------------------------------
# Slot plan: the causal lower triangle at 128-block granularity, packed into
# 512-col psum slots.  seg = (s_tile, l_off, w, pos, tri):
#   scores for s-tile `s_tile` vs l-cols [512*bank + l_off, 512*bank + l_off + w)
#   placed at slot cols [pos, pos + w); tri=True means the first 128 cols get
#   the -1e4 causal triangle accumulated on top.
# ---------------------------------------------------------------------------


def _plan_buffers():
    # flat slot stream across banks, chunked into uniform 3-slot buffers
    # (buffers may span bank boundaries -- uniform 1536-wide activations
    # keep the scalar engine saturated with no bank-edge hiccups).
    # wide banks first: dense 512-col matmul streams early keep the PE HAM
    # happy; the ragged bank 0 (one short buffer) makes a short tail.
    slots = []  # slot = [seg, ...]; seg = (bank, t, l_off, w, pos, tri)
    for j in [1, 2, 3, 0]:
        for t in range(4 * j):
            slots.append([(j, t, 0, 512, 0, False)])
        slots.append([(j, 4 * j + 0, 0, 512, 0, True)])
        slots.append(
            [(j, 4 * j + 1, 128, 384, 0, True), (j, 4 * j + 3, 384, 128, 384, True)]
        )
        slots.append([(j, 4 * j + 2, 256, 256, 0, True)])

    flat = [
        (bank, t, l_off, w, 512 * (i % SLOTS_PER_BUF) + pos, tri)
        for i, slot in enumerate(slots)
        for (bank, t, l_off, w, pos, tri) in slot
    ]
    # per-seg first/last-of-bank flags over the whole stream
    first_seen, last_idx = set(), {}
    segs = []
    for idx, (bank, t, l_off, w, p, tri) in enumerate(flat):
        first = bank not in first_seen
        first_seen.add(bank)
        last_idx[bank] = idx
        segs.append([bank, t, l_off, w, p, tri, first, False])
    for bank, idx in last_idx.items():
        segs[idx][7] = True

    buffers = []
    si = 0
    for i in range(0, len(slots), SLOTS_PER_BUF):
        grp = slots[i : i + SLOTS_PER_BUF]
        nsegs = sum(len(s) for s in grp)
        bsegs = [tuple(s) for s in segs[si : si + nsegs]]
        si += nsegs
        ncols = max(p + w for (_, _, _, w, p, _, _, _) in bsegs)
        buffers.append((bsegs, ncols))
    return buffers


# ---------------------------------------------------------------------------
# Program builder
# ---------------------------------------------------------------------------

def build_program(st_dtype=bf16, av_dtype=bf16):
    nc = bass.Bass()
    Exp = mybir.ActivationFunctionType.Exp

    qt = nc.declare_dram_parameter("qt", [PAIRS_PER_CORE, 2 * E, L], st_dtype, isOutput=False)
    kt = nc.declare_dram_parameter("kt", [PAIRS_PER_CORE, 2 * E, L], st_dtype, isOutput=False)
    vv = nc.declare_dram_parameter("vv", [PAIRS_PER_CORE, L, VW], av_dtype, isOutput=False)
    mask = nc.declare_dram_parameter("mask", [128, 128], bf16, isOutput=False)
    ident2 = nc.declare_dram_parameter("ident2", [128, E], bf16, isOutput=False)
    oo = nc.declare_dram_parameter(
        "oo", [PAIRS_PER_CORE, NB, E + 1, 512], f32, isOutput=True
    )

    buffers = _plan_buffers()

    with SplitWaitTileContext(nc) as tc:
        with (
            tc.tile_pool(name="const", bufs=1) as constp,
            tc.tile_pool(name="qk", bufs=2) as qkp,
            tc.tile_pool(name="vp", bufs=2) as vp,
            tc.tile_pool(name="ap", bufs=5) as ap_pool,
            tc.tile_pool(name="ep", bufs=2) as ep,
            tc.tile_pool(name="st", bufs=2, space="PSUM") as stp,
            tc.tile_pool(name="otp", bufs=2, space="PSUM") as otp,
        ):
            mask_sb = constp.tile([128, 128], bf16, tag="mask", name="mask_sb")
            ident2_sb = constp.tile([128, E], bf16, tag="ident2", name="ident2_sb")
            scratch = constp.tile([1, 1], f32, tag="scratch", name="scratch")

            # consts first on the sync queue (small, fast); input DMA is
            # spread over the sync / gpsimd / scalar queues (~90 GB/s each)
            # so no single queue gates the pipeline.  The scalar queue gets
            # the two earliest-needed Q chunks only, before any activations.
            nc.sync.dma_start(out=ident2_sb, in_=ident2[:])

            qt_sbs, kt_sbs, v_sbs, vv_rs = [], [], [], []
            for pair in range(PAIRS_PER_CORE):
                qt_sbs.append(qkp.tile([2 * E, L], st_dtype, tag="qt", name="qt_sb"))
                kt_sbs.append(qkp.tile([2 * E, L], st_dtype, tag="kt", name="kt_sb"))
                v_sbs.append(vp.tile([128, NT, VW], av_dtype, tag="v", name="v_sb"))
                vv_rs.append(vv[pair].rearrange("(t p) e -> p t e", p=128))

            def chunk(dst, src, ch):
                cl = slice(512 * ch, 512 * (ch + 1))
                return dict(out=dst[:, cl], in_=src[:, cl])

            # scalar queue: earliest-needed Q chunks (bank order is
            # [1,2,3,0], so qt chunk 1 gates the first buffer), then the
            # table-preload exp
            nc.scalar.dma_start(**chunk(qt_sbs[0], qt[0], 1))
            nc.scalar.dma_start(**chunk(qt_sbs[0], qt[0], 0))
            nc.vector.memset(scratch, 0.0)
            nc.scalar.activation(out=scratch, in_=scratch, func=Exp, scale=1.0)

            # sync queue: mask (gates the warm-up burst), then all K chunks
            nc.sync.dma_start(out=mask_sb, in_=mask[:])
            for ch in range(4):
                nc.sync.dma_start(**chunk(kt_sbs[0], kt[0], ch))
            for ch in range(4):
                nc.sync.dma_start(**chunk(kt_sbs[1], kt[1], ch))

            # gpsimd queue: V and remaining Q (in bank-need order)
            nc.gpsimd.dma_start(
                out=v_sbs[0][:, 0:4, :], in_=vv_rs[0][:, 0:4, :]
            )
            nc.gpsimd.dma_start(**chunk(qt_sbs[0], qt[0], 2))
            nc.gpsimd.dma_start(**chunk(qt_sbs[0], qt[0], 3))
            for ch in range(1, 4):
                nc.gpsimd.dma_start(
                    out=v_sbs[0][:, 4 * ch : 4 * ch + 4, :],
                    in_=vv_rs[0][:, 4 * ch : 4 * ch + 4, :],
                )
            for ch in [1, 2, 3, 0]:
                nc.gpsimd.dma_start(**chunk(qt_sbs[1], qt[1], ch))
                nc.gpsimd.dma_start(
                    out=v_sbs[1][:, 4 * ch : 4 * ch + 4, :],
                    in_=vv_rs[1][:, 4 * ch : 4 * ch + 4, :],
                )

            # HAM warm-up: a dense burst of back-to-back matmuls on the
            # const tiles flips the PE clock gate to 8/8 (~2.4 GHz) before
            # real work starts.  Alternating PE row halves lets each
            # LDWEIGHTS pull ahead of the other half's in-flight matmul so
            # the array itself stays busy, which is what the HAM watches.
            warm = stp.tile([128, 512 * SLOTS_PER_BUF], f32, tag="st", name="warm")
            for i in range(N_WARM):
                h = 64 * (i % 2)
                nc.tensor.matmul(
                    warm[h : h + 64, 0:128],
                    ident2_sb[h : h + 64, :],
                    mask_sb[h : h + 64, :],
                    start=True,
                    stop=True,
                )

            half_ctr = 0

            def emit_st(pair, segs, ncols):
                nonlocal half_ctr
                st = stp.tile([128, 512 * SLOTS_PER_BUF], f32, tag="st", name="st")
                qt_sb, kt_sb = qt_sbs[pair], kt_sbs[pair]
                for (bank, t, l_off, w, p, tri, _, _) in segs:
                    h = 64 * (half_ctr % 2)
                    half_ctr += 1
                    nc.tensor.matmul(
                        st[:, p : p + w],
                        kt_sb[h : h + E, 128 * t : 128 * t + 128],
                        qt_sb[h : h + E, 512 * bank + l_off : 512 * bank + l_off + w],
                        start=True,
                        stop=not tri,
                    )
                    if tri:
                        nc.tensor.matmul(
                            st[0:64, p : p + 128],
                            ident2_sb[0:64, :],
                            mask_sb[0:64, :],
                            start=False,
                            stop=True,
                        )
                        nc.tensor.matmul(
                            st[64:128, p : p + 128],
                            ident2_sb[64:128, :],
                            mask_sb[64:128, :],
                            start=False,
                            stop=True,
                        )
                a_grp = ap_pool.tile(
                    [128, 512 * SLOTS_PER_BUF], av_dtype, tag="A", name="A"
                )
                nc.scalar.activation(
                    out=a_grp[:, 0:ncols], in_=st[:, 0:ncols], func=Exp, scale=1.0
                )
                return a_grp

            ot_box = [None]

            def epilogue(pair, bank):
                # un-normalized [E+1, 512] bank straight to HBM; the host
                # divides by the denominator row and transposes
                ot = ot_box[0]
                ot_sb = ep.tile([E + 1, 512], f32, tag="ot_sb", name="ot_sb")
                nc.vector.tensor_copy(ot_sb, ot)
                nc.sync.dma_start(out=oo[pair, bank], in_=ot_sb)

            def emit_av(pair, segs, a_grp):
                v_sb = v_sbs[pair]
                for (bank, t, l_off, w, p, tri, first, last) in segs:
                    if first:
                        ot_box[0] = otp.tile([E + 1, 512], f32, tag="ot", name="ot")
                    nc.tensor.matmul(
                        ot_box[0][:, l_off : l_off + w],
                        v_sb[:, t, 0 : E + 1],
                        a_grp[:, p : p + w],
                        start=first,
                        stop=last,
                    )
                    if last:
                        epilogue(pair, bank)

            # flat pipeline across both pairs: the AV stream trails the ST
            # stream by TWO buffers so the (bottleneck) scalar engine always
            # has a finished ST buffer waiting even when the PE's AV backlog
            # slips; a_sb is 4-deep to cover the extra lag
            work = [
                (pair, segs, ncols)
                for pair in range(PAIRS_PER_CORE)
                for (segs, ncols) in buffers
            ]
            pending = []
            for (pair, segs, ncols) in work:
                a_grp = emit_st(pair, segs, ncols)
                pending.append((pair, segs, a_grp))
                if len(pending) > 4:
                    emit_av(*pending.pop(0))
            for item in pending:
                emit_av(*item)

    return nc


# ---------------------------------------------------------------------------
# Host-side sharding / unsharding
# ---------------------------------------------------------------------------

def _in_maps(queries, keys, values, tau, delta, st_dtype=bf16, av_dtype=bf16):
    np_st = mybir.dt.np(st_dtype)
    np_av = mybir.dt.np(av_dtype)
    sl, ll = np.meshgrid(np.arange(128), np.arange(128), indexing="ij")
    mask = np.where(sl > ll, np.float32(NEG), np.float32(0.0)).astype(
        mybir.dt.np(bf16)
    )
    ident2 = np.concatenate([np.eye(E, dtype=np.float32)] * 2, axis=0).astype(
        mybir.dt.np(bf16)
    )
    maps = []
    for c in range(N_CORES):
        ps = [2 * c, 2 * c + 1]
        b = ps[0] // H
        hs = [p % H for p in ps]
        qscale = np.float32(SCALE * tau[b, 0])
        qt = np.ascontiguousarray(
            np.stack(
                [np.tile(queries[b, :, h, :].T * qscale, (2, 1)) for h in hs]
            )
        ).astype(np_st)
        kt = np.ascontiguousarray(
            np.stack([np.tile(keys[b, :, h, :].T, (2, 1)) for h in hs])
        ).astype(np_st)
        # V augmented with the delta fold: cols 0..63 = V * exp(delta'),
        # col 64 = exp(delta') (denominator), col 65 pad
        expd = np.exp(SCALE * delta[b]).astype(np.float32)  # [L]
        vvv = np.zeros((PAIRS_PER_CORE, L, VW), dtype=np.float32)
        for i, h in enumerate(hs):
            vvv[i, :, 0:E] = values[b, :, h, :] * expd[:, None]
            vvv[i, :, E] = expd
        vvv = np.ascontiguousarray(vvv).astype(np_av)
        maps.append(
            {"qt": qt, "kt": kt, "vv": vvv, "mask": mask, "ident2": ident2}
        )
    return maps


_CACHED = {}


def run(queries, keys, values, tau, delta, trace=False, st_dtype=bf16,
        av_dtype=bf16):
    key = (str(st_dtype), str(av_dtype))
    if key not in _CACHED:
        _CACHED[key] = build_program(st_dtype, av_dtype)
    nc = _CACHED[key]
    in_maps = _in_maps(
        np.asarray(queries),
        np.asarray(keys),
        np.asarray(values),
        np.asarray(tau),
        np.asarray(delta),
        st_dtype=st_dtype,
        av_dtype=av_dtype,
    )
    res = run_bass_kernel_spmd(
        nc, in_maps, core_ids=list(range(N_CORES)), trace=trace
    )
    out = np.empty((B, L, H, E), dtype=np.float32)
    for c in range(N_CORES):
        o = res.results[c]["oo"]  # [PAIRS, NB, E+1, 512]
        # host epilogue: normalize by the denominator row and transpose
        num = o[:, :, 0:E, :]  # [PAIRS, NB, E, 512]
        den = o[:, :, E : E + 1, :]  # [PAIRS, NB, 1, 512]
        norm = (num / den).transpose(0, 1, 3, 2).reshape(PAIRS_PER_CORE, L, E)
        for i, p in enumerate([2 * c, 2 * c + 1]):
            out[p // H, :, p % H, :] = norm[i]
    return out, res


def kernel(queries, keys, values, tau, delta):
    out, _ = run(queries, keys, values, tau, delta, trace=False)
    return out


# revision 27
# speedup vs baseline: 1.0143x; 1.0143x over previous
"""De-stationary causal attention (B=2, L=S=2048, H=8, E=64) on 8 TRN2 cores.

Sharding: the 16 (batch, head) pairs are distributed 2-per-core (cores 0-3
get batch 0, heads 0..7; cores 4-7 get batch 1). Each core runs the same
Bass program on its two pairs.

Math: logits = (Q K^T) * (tau/sqrt(E)) + delta/sqrt(E), causal softmax, A V.
Host-side folds: Q is pre-scaled by tau/sqrt(E); exp(delta/sqrt(E)) is folded
into V (and into the appended denominator column), because
softmax(x + d)_s = exp(x_s) e^{d_s} / sum_j exp(x_j) e^{d_j}.

Device structure per (b,h) pair, scores kept TRANSPOSED (s on partitions):
the causal lower triangle is tiled into 128(s)x{128..512}(l) chunks packed
into 512-col PSUM "slots" (1 bank each).  Slots are grouped 3 per ST buffer
([128,1536] = 3 banks, double buffered) and each buffer gets ONE big exp
on the scalar engine -- the ACT engine is the roofline for this kernel, so
activation count/width is what everything else is shaped around.  The
causal triangle inside diagonal 128-blocks is handled by ACCUMULATING a
-1e4 triangle into PSUM right after the score matmul (two concurrent k=64
identity matmuls on partition halves, bf16), so exp() underflows to exactly
0 and no element-wise masking is needed anywhere.  AV matmuls then stream
the packed A segments into a per-bank [65,512] PSUM accumulator (col 64 is
the softmax denominator, via a ones-ish column folded into V).  Each bank
is copied to SBUF and DMA'd out UN-normalized and transposed; the host does
out[l,e] = ot[e,l] / ot[64,l] (O(L*E) work, off the device critical path).

A dense burst of back-to-back matmuls at the start (alternating PE row
halves so LDWEIGHTS pipelines) flips the PE HAM clock gate to 2.4 GHz
before real work begins.
"""

import copy
import sys

import numpy as np

try:
    import concourse.bass as bass
except ImportError:  # pragma: no cover
    sys.path.insert(0, "/opt/trn_rl_repo")
    import concourse.bass as bass

import concourse.mybir as mybir
import concourse.tile as tile
from concourse.bass_utils import run_bass_kernel_spmd
from concourse.vector_clock import ScopedClock

B, L, H, E = 2, 2048, 8, 64
N_CORES = 8
PAIRS_PER_CORE = 2
SCALE = 1.0 / np.sqrt(np.float32(E))  # 0.125
NEG = -1.0e4  # causal-mask addend; exp(NEG + score) == 0.0 in fp32

f32 = mybir.dt.float32
f32r = mybir.dt.float32r
bf16 = mybir.dt.bfloat16

NT = L // 128  # 16 s-tiles / l-tiles
NB = L // 512  # 4 output banks of 512 l-values
VW = E + 2  # v row: 64 values + denominator col + pad
SLOTS_PER_BUF = 3  # 3 psum banks per ST buffer
N_WARM = 12  # HAM warm-up matmuls

# ---------------------------------------------------------------------------
# Walrus in this toolchain rejects >1 sync-wait per instruction. Split extra
# waits onto NoOps committed just before the instruction on the same engine.
# ---------------------------------------------------------------------------
_NOP_TEMPLATE = {}


def _make_nop(engine, name):
    if engine not in _NOP_TEMPLATE:
        tmp = bass.Bass()
        _NOP_TEMPLATE[engine] = tmp.engines[engine].nop(nofuse=True).ins
    nop = copy.copy(_NOP_TEMPLATE[engine])
    nop.name = name
    nop.engine = engine
    nop.sync_info = None
    return nop


class SplitWaitTileContext(tile.TileContext):
    _ws_counter = 0

    def _split_waits(self, inst):
        si = inst.sync_info
        if si is None or not si.on_wait or len(si.on_wait) <= 1:
            return []
        if inst.engine == mybir.EngineType.Unassigned:
            return []
        waits = list(si.on_wait)
        inst.sync_info = mybir.SyncInfo(
            on_wait=[waits[0]], on_update=list(si.on_update or [])
        )
        nops = []
        for w in waits[1:]:
            SplitWaitTileContext._ws_counter += 1
            nop = _make_nop(inst.engine, f"I-ws{SplitWaitTileContext._ws_counter}")
            nop.sync_info = mybir.SyncInfo(on_wait=[w], on_update=[])
            nops.append(nop)
        return nops

    def _commit_instruction(self, inst, lazy_reg_writes=True):
        for nop in self._split_waits(inst):
            self._add_instruction(nop)
        super()._commit_instruction(inst, lazy_reg_writes)

    def _drain_and_barrier(self, tick_clock, wait_clock):
        nc = self.nc
        probe = nc.sync.nop(nofuse=True)
        wait_clock.add_sem_waits(
            probe.ins, ScopedClock({None: tick_clock.global_clock})
        )
        waits = list(probe.ins.sync_info.on_wait or []) if probe.ins.sync_info else []
        if len(waits) > 1:
            probe.ins.sync_info.on_wait = [waits[0]]
            handles = {h.num: h for h in self.sems.allocated().values()}
            for w in waits[1:]:
                nop = nc.sync.nop(nofuse=True)
                nop.wait_op(handles[w.id], w.wait_value, "sem-ge")
        nc.sync.drain()

        nc.all_engine_barrier()
        assert self.sems is not None
        popped = nc._tile_sem_poison_stack.pop()
        assert popped is self._sem_poison
        nc.clear_and_free_semaphores(list(self.sems.allocated().values()))
        nc.all_engine_barrier()


# ---------------------------------------------------------------------------
# Slot plan: the causal lower triangle at 128-block granularity, packed into
# 512-col psum slots.  seg = (s_tile, l_off, w, pos, tri):
#   scores for s-tile `s_tile` vs l-cols [512*bank + l_off, 512*bank + l_off + w)
#   placed at slot cols [pos, pos + w); tri=True means the first 128 cols get
#   the -1e4 causal triangle accumulated on top.
# ---------------------------------------------------------------------------


def _plan_buffers():
    # flat slot stream across banks, chunked into uniform 3-slot buffers
    # (buffers may span bank boundaries -- uniform 1536-wide activations
    # keep the scalar engine saturated with no bank-edge hiccups).
    # wide banks first: dense 512-col matmul streams early keep the PE HAM
    # happy; the ragged bank 0 (one short buffer) makes a short tail.
    slots = []  # slot = [seg, ...]; seg = (bank, t, l_off, w, pos, tri)
    for j in [1, 2, 3, 0]:
        for t in range(4 * j):
            slots.append([(j, t, 0, 512, 0, False)])
        slots.append([(j, 4 * j + 0, 0, 512, 0, True)])
        slots.append(
            [(j, 4 * j + 1, 128, 384, 0, True), (j, 4 * j + 3, 384, 128, 384, True)]
        )
        slots.append([(j, 4 * j + 2, 256, 256, 0, True)])

    flat = [
        (bank, t, l_off, w, 512 * (i % SLOTS_PER_BUF) + pos, tri)
        for i, slot in enumerate(slots)
        for (bank, t, l_off, w, pos, tri) in slot
    ]
    # per-seg first/last-of-bank flags over the whole stream
    first_seen, last_idx = set(), {}
    segs = []
    for idx, (bank, t, l_off, w, p, tri) in enumerate(flat):
        first = bank not in first_seen
        first_seen.add(bank)
        last_idx[bank] = idx
        segs.append([bank, t, l_off, w, p, tri, first, False])
    for bank, idx in last_idx.items():
        segs[idx][7] = True

    buffers = []
    si = 0
    for i in range(0, len(slots), SLOTS_PER_BUF):
        grp = slots[i : i + SLOTS_PER_BUF]
        nsegs = sum(len(s) for s in grp)
        bsegs = [tuple(s) for s in segs[si : si + nsegs]]
        si += nsegs
        ncols = max(p + w for (_, _, _, w, p, _, _, _) in bsegs)
        buffers.append((bsegs, ncols))
    return buffers


# ---------------------------------------------------------------------------
# Program builder
# ---------------------------------------------------------------------------

def build_program(st_dtype=bf16, av_dtype=bf16):
    nc = bass.Bass()
    Exp = mybir.ActivationFunctionType.Exp

    qt = nc.declare_dram_parameter("qt", [PAIRS_PER_CORE, 2 * E, L], st_dtype, isOutput=False)
    kt = nc.declare_dram_parameter("kt", [PAIRS_PER_CORE, 2 * E, L], st_dtype, isOutput=False)
    vv = nc.declare_dram_parameter("vv", [PAIRS_PER_CORE, L, VW], av_dtype, isOutput=False)
    mask = nc.declare_dram_parameter("mask", [128, 128], bf16, isOutput=False)
    ident2 = nc.declare_dram_parameter("ident2", [128, E], bf16, isOutput=False)
    oo = nc.declare_dram_parameter(
        "oo", [PAIRS_PER_CORE, NB, E + 1, 512], f32, isOutput=True
    )

    buffers = _plan_buffers()

    with SplitWaitTileContext(nc) as tc:
        with (
            tc.tile_pool(name="const", bufs=1) as constp,
            tc.tile_pool(name="qk", bufs=2) as qkp,
            tc.tile_pool(name="vp", bufs=2) as vp,
            tc.tile_pool(name="ap", bufs=4) as ap_pool,
            tc.tile_pool(name="ep", bufs=2) as ep,
            tc.tile_pool(name="st", bufs=2, space="PSUM") as stp,
            tc.tile_pool(name="otp", bufs=2, space="PSUM") as otp,
        ):
            mask_sb = constp.tile([128, 128], bf16, tag="mask", name="mask_sb")
            ident2_sb = constp.tile([128, E], bf16, tag="ident2", name="ident2_sb")
            scratch = constp.tile([1, 1], f32, tag="scratch", name="scratch")

            # consts first on the sync queue (small, fast); input DMA is
            # spread over the sync / gpsimd / scalar queues (~90 GB/s each)
            # so no single queue gates the pipeline.  The scalar queue gets
            # the two earliest-needed Q chunks only, before any activations.
            nc.sync.dma_start(out=ident2_sb, in_=ident2[:])

            qt_sbs, kt_sbs, v_sbs, vv_rs = [], [], [], []
            for pair in range(PAIRS_PER_CORE):
                qt_sbs.append(qkp.tile([2 * E, L], st_dtype, tag="qt", name="qt_sb"))
                kt_sbs.append(qkp.tile([2 * E, L], st_dtype, tag="kt", name="kt_sb"))
                v_sbs.append(vp.tile([128, NT, VW], av_dtype, tag="v", name="v_sb"))
                vv_rs.append(vv[pair].rearrange("(t p) e -> p t e", p=128))

            def chunk(dst, src, ch):
                cl = slice(512 * ch, 512 * (ch + 1))
                return dict(out=dst[:, cl], in_=src[:, cl])

            # scalar queue: earliest-needed Q chunks (bank order is
            # [1,2,3,0], so qt chunk 1 gates the first buffer), then the
            # table-preload exp
            nc.scalar.dma_start(**chunk(qt_sbs[0], qt[0], 1))
            nc.scalar.dma_start(**chunk(qt_sbs[0], qt[0], 0))
            nc.vector.memset(scratch, 0.0)
            nc.scalar.activation(out=scratch, in_=scratch, func=Exp, scale=1.0)

            # sync queue: mask (gates the warm-up burst), then all K chunks
            nc.sync.dma_start(out=mask_sb, in_=mask[:])
            for ch in range(4):
                nc.sync.dma_start(**chunk(kt_sbs[0], kt[0], ch))
            for ch in range(4):
                nc.sync.dma_start(**chunk(kt_sbs[1], kt[1], ch))

            # gpsimd queue: V and remaining Q (in bank-need order)
            nc.gpsimd.dma_start(
                out=v_sbs[0][:, 0:4, :], in_=vv_rs[0][:, 0:4, :]
            )
            nc.gpsimd.dma_start(**chunk(qt_sbs[0], qt[0], 2))
            nc.gpsimd.dma_start(**chunk(qt_sbs[0], qt[0], 3))
            for ch in range(1, 4):
                nc.gpsimd.dma_start(
                    out=v_sbs[0][:, 4 * ch : 4 * ch + 4, :],
                    in_=vv_rs[0][:, 4 * ch : 4 * ch + 4, :],
                )
            for ch in [1, 2, 3, 0]:
                nc.gpsimd.dma_start(**chunk(qt_sbs[1], qt[1], ch))
                nc.gpsimd.dma_start(
                    out=v_sbs[1][:, 4 * ch : 4 * ch + 4, :],
                    in_=vv_rs[1][:, 4 * ch : 4 * ch + 4, :],
                )

            # HAM warm-up: a dense burst of back-to-back matmuls on the
            # const tiles flips the PE clock gate to 8/8 (~2.4 GHz) before
            # real work starts.  Alternating PE row halves lets each
            # LDWEIGHTS pull ahead of the other half's in-flight matmul so
            # the array itself stays busy, which is what the HAM watches.
            warm = stp.tile([128, 512 * SLOTS_PER_BUF], f32, tag="st", name="warm")
            for i in range(N_WARM):
                h = 64 * (i % 2)
                nc.tensor.matmul(
                    warm[h : h + 64, 0:128],
                    ident2_sb[h : h + 64, :],
                    mask_sb[h : h + 64, :],
                    start=True,
                    stop=True,
                )

            half_ctr = 0

            def emit_st(pair, segs, ncols):
                nonlocal half_ctr
                st = stp.tile([128, 512 * SLOTS_PER_BUF], f32, tag="st", name="st")
                qt_sb, kt_sb = qt_sbs[pair], kt_sbs[pair]
                for (bank, t, l_off, w, p, tri, _, _) in segs:
                    h = 64 * (half_ctr % 2)
                    half_ctr += 1
                    nc.tensor.matmul(
                        st[:, p : p + w],
                        kt_sb[h : h + E, 128 * t : 128 * t + 128],
                        qt_sb[h : h + E, 512 * bank + l_off : 512 * bank + l_off + w],
                        start=True,
                        stop=not tri,
                    )
                    if tri:
                        nc.tensor.matmul(
                            st[0:64, p : p + 128],
                            ident2_sb[0:64, :],
                            mask_sb[0:64, :],
                            start=False,
                            stop=True,
                        )
                        nc.tensor.matmul(
                            st[64:128, p : p + 128],
                            ident2_sb[64:128, :],
                            mask_sb[64:128, :],
                            start=False,
                            stop=True,
                        )
                a_grp = ap_pool.tile(
                    [128, 512 * SLOTS_PER_BUF], av_dtype, tag="A", name="A"
                )
                nc.scalar.activation(
                    out=a_grp[:, 0:ncols], in_=st[:, 0:ncols], func=Exp, scale=1.0
                )
                return a_grp

            ot_box = [None]

            def epilogue(pair, bank):
                # un-normalized [E+1, 512] bank straight to HBM; the host
                # divides by the denominator row and transposes
                ot = ot_box[0]
                ot_sb = ep.tile([E + 1, 512], f32, tag="ot_sb", name="ot_sb")
                nc.vector.tensor_copy(ot_sb, ot)
                nc.sync.dma_start(out=oo[pair, bank], in_=ot_sb)

            def emit_av(pair, segs, a_grp):
                v_sb = v_sbs[pair]
                for (bank, t, l_off, w, p, tri, first, last) in segs:
                    if first:
                        ot_box[0] = otp.tile([E + 1, 512], f32, tag="ot", name="ot")
                    nc.tensor.matmul(
                        ot_box[0][:, l_off : l_off + w],
                        v_sb[:, t, 0 : E + 1],
                        a_grp[:, p : p + w],
                        start=first,
                        stop=last,
                    )
                    if last:
                        epilogue(pair, bank)

            # flat pipeline across both pairs: the AV stream trails the ST
            # stream by TWO buffers so the (bottleneck) scalar engine always
            # has a finished ST buffer waiting even when the PE's AV backlog
            # slips; a_sb is 4-deep to cover the extra lag
            work = [
                (pair, segs, ncols)
                for pair in range(PAIRS_PER_CORE)
                for (segs, ncols) in buffers
            ]
            pending = []
            for (pair, segs, ncols) in work:
                a_grp = emit_st(pair, segs, ncols)
                pending.append((pair, segs, a_grp))
                if len(pending) > 3:
                    emit_av(*pending.pop(0))
            for item in pending:
                emit_av(*item)

    return nc


# ---------------------------------------------------------------------------
# Host-side sharding / unsharding
# ---------------------------------------------------------------------------

def _in_maps(queries, keys, values, tau, delta, st_dtype=bf16, av_dtype=bf16):
    np_st = mybir.dt.np(st_dtype)
    np_av = mybir.dt.np(av_dtype)
    sl, ll = np.meshgrid(np.arange(128), np.arange(128), indexing="ij")
    mask = np.where(sl > ll, np.float32(NEG), np.float32(0.0)).astype(
        mybir.dt.np(bf16)
    )
    ident2 = np.concatenate([np.eye(E, dtype=np.float32)] * 2, axis=0).astype(
        mybir.dt.np(bf16)
    )
    maps = []
    for c in range(N_CORES):
        ps = [2 * c, 2 * c + 1]
        b = ps[0] // H
        hs = [p % H for p in ps]
        qscale = np.float32(SCALE * tau[b, 0])
        qt = np.ascontiguousarray(
            np.stack(
                [np.tile(queries[b, :, h, :].T * qscale, (2, 1)) for h in hs]
            )
        ).astype(np_st)
        kt = np.ascontiguousarray(
            np.stack([np.tile(keys[b, :, h, :].T, (2, 1)) for h in hs])
        ).astype(np_st)
        # V augmented with the delta fold: cols 0..63 = V * exp(delta'),
        # col 64 = exp(delta') (denominator), col 65 pad
        expd = np.exp(SCALE * delta[b]).astype(np.float32)  # [L]
        vvv = np.zeros((PAIRS_PER_CORE, L, VW), dtype=np.float32)
        for i, h in enumerate(hs):
            vvv[i, :, 0:E] = values[b, :, h, :] * expd[:, None]
            vvv[i, :, E] = expd
        vvv = np.ascontiguousarray(vvv).astype(np_av)
        maps.append(
            {"qt": qt, "kt": kt, "vv": vvv, "mask": mask, "ident2": ident2}
        )
    return maps


_CACHED = {}


def run(queries, keys, values, tau, delta, trace=False, st_dtype=bf16,
        av_dtype=bf16):
    key = (str(st_dtype), str(av_dtype))
    if key not in _CACHED:
        _CACHED[key] = build_program(st_dtype, av_dtype)
    nc = _CACHED[key]
    in_maps = _in_maps(
        np.asarray(queries),
        np.asarray(keys),
        np.asarray(values),
        np.asarray(tau),
        np.asarray(delta),
        st_dtype=st_dtype,
        av_dtype=av_dtype,
    )
    res = run_bass_kernel_spmd(
        nc, in_maps, core_ids=list(range(N_CORES)), trace=trace
    )
    out = np.empty((B, L, H, E), dtype=np.float32)
    for c in range(N_CORES):
        o = res.results[c]["oo"]  # [PAIRS, NB, E+1, 512]
        # host epilogue: normalize by the denominator row and transpose
        num = o[:, :, 0:E, :]  # [PAIRS, NB, E, 512]
        den = o[:, :, E : E + 1, :]  # [PAIRS, NB, 1, 512]
        norm = (num / den).transpose(0, 1, 3, 2).reshape(PAIRS_PER_CORE, L, E)
        for i, p in enumerate([2 * c, 2 * c + 1]):
            out[p // H, :, p % H, :] = norm[i]
    return out, res


def kernel(queries, keys, values, tau, delta):
    out, _ = run(queries, keys, values, tau, delta, trace=False)
    return out
